# revision 15
# baseline (speedup 1.0000x reference)
"""Trainium2 Bass kernel for LoRA multi-head causal attention (tensor-parallel
over heads across 8 NeuronCores).

Math (per reference):
  q = x@wq + (x@wq_A)@wq_B * 2 ; k,v analogous ; rope(q,k) ; causal softmax
  attention ; out = a@wo + (a@wo_A)@wo_B * 2

Device strategy (per core c, heads 2c and 2c+1):
  - LoRA folded into the dense weights on host (x@W + (x@A)@B*s == x@(W+s*A@B)).
  - q/k weights column-permuted per head (even rope pairs first) so rope is
    contiguous half-tile arithmetic; QK^T is invariant under a shared head-dim
    permutation.
  - Matmul operands in bf16; accumulation and softmax arithmetic in fp32.
  - Phase 1: Q^T,K^T (head-dim on partitions) + V (natural) from xT = x^T
    staged in DRAM; rope applied PSUM->SBUF.
  - Phase 2: S^T = K^T.T @ Q^T per (batch,head,q-tile); the two heads of a
    q-tile are emitted INTERLEAVED so the PE always has the other head's
    matmuls while one head's exp is on ScalarE.  exp on ScalarE (pair of
    k-blocks per instruction, scale=1/sqrt(hd)); multiplicative exp(mask)
    tiles on partially-masked blocks; P^T feeds PV and a ones-vector rowsum
    matmul.  Both heads' rowsums share ONE psum bank (h0 at partition 0 via
    start=True clear, h1 rides the clear at partition 32).  Normalization:
    DVE reciprocal -> GpSimd partition_broadcast -> one fused DVE multiply
    (po * rb -> OT, bf16).  wo projection emitted deferred, overlapping the
    next q-tile's attention; output written bf16.
  - Host sums the per-core bf16 partials in fp32 (the tensor-parallel
    all-reduce).
"""
import sys
import math

sys.path.insert(0, "/opt/trn_rl_repo")

import numpy as np
import ml_dtypes

import concourse.bass as bass
from concourse import bacc
import concourse.mybir as mybir
from concourse.tile import TileContext
from concourse.bass_utils import run_bass_kernel_spmd

F32 = mybir.dt.float32
F32R = mybir.dt.float32r
BF16 = mybir.dt.bfloat16

B, S, D, H, R = 2, 2048, 2048, 16, 8
HD = D // H                     # 128
SCALING = 16.0 / R              # 2.0
N_CORES = 8
HPC = H // N_CORES              # heads per core = 2
DCOL = HPC * HD                 # per-core projection width = 256
SEQ = B * S                     # 4096
ISQ = 1.0 / math.sqrt(HD)
QTILE = 512                     # q-tile width (free dim)
KBLK = 128                      # k-block (partition dim)


def build_kernel(blocks, nm, kc, nsb, nqt, nkb):
    """blocks: per q-tile (within a batch) list of (kt, mask_id|None)."""
    nc = bacc.Bacc("TRN2", target_bir_lowering=False, debug=False)

    xT = nc.declare_dram_parameter("xT", [D, SEQ], BF16, isOutput=False)
    wq = nc.declare_dram_parameter("wq", [D, DCOL], BF16, isOutput=False)
    wk = nc.declare_dram_parameter("wk", [D, DCOL], BF16, isOutput=False)
    wv = nc.declare_dram_parameter("wv", [D, DCOL], BF16, isOutput=False)
    wo = nc.declare_dram_parameter("wo", [DCOL, D], BF16, isOutput=False)
    cos2 = nc.declare_dram_parameter("cos2", [HD, S], F32, isOutput=False)
    sin2 = nc.declare_dram_parameter("sin2", [HD, S], F32, isOutput=False)
    masks = nc.declare_dram_parameter("masks", [nm * 128, QTILE], BF16, isOutput=False)
    ones = nc.declare_dram_parameter("ones", [128, 1], BF16, isOutput=False)
    onesrow = nc.declare_dram_parameter("onesrow", [1, 128], F32R, isOutput=False)
    outT = nc.declare_dram_parameter("outT", [D, SEQ], BF16, isOutput=True)

    with TileContext(nc) as tc:
        from contextlib import ExitStack
        with ExitStack() as top:
            glob = top.enter_context(tc.tile_pool(name="glob", bufs=1))
            qkvs = top.enter_context(tc.tile_pool(name="qkvs", bufs=1))

            QT = qkvs.tile([128, HPC, nsb, QTILE], BF16, tag="QT")
            KT = qkvs.tile([128, HPC, nsb, QTILE], BF16, tag="KT")
            VS = qkvs.tile([128, SEQ // 128, DCOL], BF16, tag="VS")
            OT = qkvs.tile([128, HPC, nsb, QTILE], BF16, tag="OT")

            # ---------------- Phase 1: projections + rope ----------------
            with tc.tile_pool(name="wts", bufs=1) as wts, \
                 tc.tile_pool(name="xts", bufs=8) as xts, \
                 tc.tile_pool(name="tmp", bufs=3) as tmp, \
                 tc.tile_pool(name="ps1q", bufs=2, space="PSUM") as ps1q, \
                 tc.tile_pool(name="ps1", bufs=1, space="PSUM") as ps1:
                wq_t = wts.tile([128, kc, DCOL], BF16, tag="wq")
                wk_t = wts.tile([128, kc, DCOL], BF16, tag="wk")
                wv_t = wts.tile([128, kc, DCOL], BF16, tag="wv")

                sb_order = [x for pair in zip(range(nsb // 2), range(nsb // 2, nsb))
                            for x in pair] if nsb % 2 == 0 else list(range(nsb))
                for sbi, sb in enumerate(sb_order):
                    last_sb = sbi == len(sb_order) - 1
                    # start=True clears the WHOLE psum bank -> one chain per
                    # psum tensor, EXCEPT the deliberate V bank-share below:
                    # the second region never uses start and rides the first
                    # region's bank clear (its has_written bits stay 0 until
                    # its own first write).
                    q_ps = [ps1q.tile([128, QTILE], F32, tag=f"q{h}", name=f"q_ps{h}") for h in range(HPC)]
                    k_ps = [ps1.tile([128, QTILE], F32, tag=f"k{h}", name=f"k_ps{h}") for h in range(HPC)]
                    v_ps = [ps1.tile([128, 2, DCOL], F32, tag=f"v{j}", name=f"v_ps{j}")
                            for j in range(2)]
                    if sb == 0:
                        # wq on the Sync queue (first use), wv/wk/cos/sin on
                        # the GpSimd software-DGE queue so the two DMA streams
                        # issue in parallel and sb0 is not Sync-issue-bound.
                        nc.sync.dma_start(
                            out=wq_t[:, 0:2, :],
                            in_=wq[0:256, :].rearrange("(c p) d -> p c d", c=2))
                        nc.sync.dma_start(
                            out=wq_t[:, 2:kc, :],
                            in_=wq[256:, :].rearrange("(c p) d -> p c d", c=kc - 2))
                        nc.gpsimd.dma_start(
                            out=wv_t, in_=wv[:, :].rearrange("(c p) d -> p c d", c=kc))
                        nc.gpsimd.dma_start(
                            out=wk_t, in_=wk[:, :].rearrange("(c p) d -> p c d", c=kc))
                        cos2_t = glob.tile([HD, S], F32, tag="cos2")
                        nc.gpsimd.dma_start(out=cos2_t, in_=cos2[:, :])
                        sin2_t = glob.tile([HD, S], F32, tag="sin2")
                        nc.gpsimd.dma_start(out=sin2_t, in_=sin2[:, :])
                    if not last_sb:
                        for c in range(kc):
                            xt = xts.tile([128, QTILE], BF16, tag="xt")
                            nc.sync.dma_start(
                                out=xt, in_=xT[c * 128:(c + 1) * 128, sb * QTILE:(sb + 1) * QTILE])
                            st, sp = (c == 0), (c == kc - 1)
                            for h in range(HPC):
                                nc.tensor.matmul(q_ps[h], wq_t[:, c, h * HD:(h + 1) * HD], xt,
                                                 start=st, stop=sp)
                            for sub in range(4):
                                nc.tensor.matmul(v_ps[sub // 2][:, sub % 2, :],
                                                 xt[:, sub * 128:(sub + 1) * 128],
                                                 wv_t[:, c, :],
                                                 start=(st and sub % 2 == 0), stop=sp,
                                                 skip_group_check=True)
                            for h in range(HPC):
                                nc.tensor.matmul(k_ps[h], wk_t[:, c, h * HD:(h + 1) * HD], xt,
                                                 start=st, stop=sp)
                    else:
                        # last sb: pass-ordered (Q then V then K) with the q
                        # ropes emitted mid-sb, so phase 2's score psum banks
                        # (reused q banks) are free BEFORE the projections end
                        # and the first QK matmuls follow with no PE gap.
                        xtiles = []
                        for c in range(kc):
                            xt = xts.tile([128, QTILE], BF16, tag="xtl", bufs=kc,
                                          name="xtl")
                            nc.sync.dma_start(
                                out=xt, in_=xT[c * 128:(c + 1) * 128, sb * QTILE:(sb + 1) * QTILE])
                            xtiles.append(xt)
                        for c in range(kc):
                            for h in range(HPC):
                                nc.tensor.matmul(q_ps[h], wq_t[:, c, h * HD:(h + 1) * HD],
                                                 xtiles[c],
                                                 start=(c == 0), stop=(c == kc - 1))
                    # rope: psum [e;o] rows -> [re;im] rows in SBUF (bf16).
                    # ps*sin computed in place; half-cross combines rely on
                    # mixed SBUF/PSUM operands allowing different bases.
                    scol = (sb * QTILE) % S
                    cs = cos2_t[:, scol:scol + QTILE]
                    sn = sin2_t[:, scol:scol + QTILE]
                    def rope(h, ps, dst):
                        t1 = tmp.tile([128, QTILE], F32, tag="t1", name="t1")
                        nc.vector.tensor_mul(t1, ps, cs)
                        nc.vector.tensor_mul(ps, ps, sn)
                        nc.vector.tensor_sub(dst[0:64, h, sb, :], t1[0:64, :], ps[64:128, :])
                        nc.vector.tensor_add(dst[64:128, h, sb, :], ps[0:64, :], t1[64:128, :])

                    if not last_sb:
                        # V copies first (free the shared banks the next sb's
                        # V matmuls need), then rope.
                        for j in range(2):
                            nc.vector.tensor_copy(
                                VS[:, sb * 4 + 2 * j: sb * 4 + 2 * j + 2, :].rearrange("p a b -> p (a b)"),
                                v_ps[j].rearrange("p a b -> p (a b)"))
                        for h in range(HPC):
                            rope(h, k_ps[h], KT)
                        for h in range(HPC):
                            rope(h, q_ps[h], QT)
                    else:
                        # q ropes overlap the V-pass; V copies overlap the
                        # K-pass; k ropes are the only post-projection work.
                        for h in range(HPC):
                            rope(h, q_ps[h], QT)
                        for c in range(kc):
                            for sub in range(4):
                                nc.tensor.matmul(v_ps[sub // 2][:, sub % 2, :],
                                                 xtiles[c][:, sub * 128:(sub + 1) * 128],
                                                 wv_t[:, c, :],
                                                 start=(c == 0 and sub % 2 == 0),
                                                 stop=(c == kc - 1),
                                                 skip_group_check=True)
                        for j in range(2):
                            nc.vector.tensor_copy(
                                VS[:, sb * 4 + 2 * j: sb * 4 + 2 * j + 2, :].rearrange("p a b -> p (a b)"),
                                v_ps[j].rearrange("p a b -> p (a b)"))
                        for c in range(kc):
                            for h in range(HPC):
                                nc.tensor.matmul(k_ps[h], wk_t[:, c, h * HD:(h + 1) * HD],
                                                 xtiles[c],
                                                 start=(c == 0), stop=(c == kc - 1))
                        for h in range(HPC):
                            rope(h, k_ps[h], KT)

            # ---------------- Phase 2 + 3 interleaved ----------------
            # PSUM bank budget (8): ps_s 2x[128,2,Q]=4 (scores AND wo pairs),
            # po0/po1 1 each, pr0/pr1 1 each.
            with tc.tile_pool(name="ps_s", bufs=2, space="PSUM") as ps_s, \
                 tc.tile_pool(name="ps_o", bufs=1, space="PSUM") as ps_o, \
                 tc.tile_pool(name="ps_r", bufs=1, space="PSUM") as ps_r, \
                 tc.tile_pool(name="wos", bufs=1) as wos, \
                 tc.tile_pool(name="pts", bufs=6) as pts, \
                 tc.tile_pool(name="rbs", bufs=2) as rbs, \
                 tc.tile_pool(name="outs", bufs=4) as outs:
                wo_t = wos.tile([128, HPC, D], BF16, tag="wo")
                for j in range(HPC):
                    nc.sync.dma_start(out=wo_t[:, j, :], in_=wo[j * 128:(j + 1) * 128, :])
                ones_t = glob.tile([128, 1], BF16, tag="ones")
                nc.sync.dma_start(out=ones_t, in_=ones[:, :])
                onesr_t = glob.tile([1, 128], F32R, tag="onesr")
                nc.sync.dma_start(out=onesr_t, in_=onesrow[:, :])
                masks_t = glob.tile([128, nm, QTILE], BF16, tag="masks")
                for m in range(nm):
                    nc.sync.dma_start(out=masks_t[:, m, :], in_=masks[m * 128:(m + 1) * 128, :])

                # Deferred-op scheduler: normalization / wo projection pieces
                # are emitted a few chain-steps after their inputs start, so
                # the in-order PE stream never waits on them.
                import heapq
                todo = []      # (due_step, seq, fn)
                gstep = 0
                seq = [0]

                def sched(delay, fn):
                    heapq.heappush(todo, (gstep + delay, seq[0], fn))
                    seq[0] += 1

                def emit_due():
                    while todo and todo[0][0] <= gstep:
                        heapq.heappop(todo)[2]()

                def wo_part(sbq, dc0, ndc, tail=False):
                    # wo chunk pairs rotate through the score psum pool, so wo
                    # accumulation is double-buffered against the os2 copies.
                    # At the very end of the kernel (tail=True) the attention
                    # accumulator banks are free: odd pairs use them, with the
                    # evacuation copies on ScalarE, so four wo chains overlap.
                    for dpi, dp in enumerate(range(dc0, dc0 + ndc, 2)):
                        if tail and dpi % 2 == 1:
                            p3 = [ps_o.tile([128, QTILE], F32, tag=f"po{h}",
                                            name=f"p3{h}", bufs=1)
                                  for h in range(2)]
                            for jj in range(2):
                                dc = dp + jj
                                for j in range(HPC):
                                    nc.tensor.matmul(p3[jj],
                                                     wo_t[:, j, dc * 128:(dc + 1) * 128],
                                                     OT[:, j, sbq, :],
                                                     start=(j == 0), stop=(j == HPC - 1))
                            os2 = outs.tile([128, 2, QTILE], BF16, tag="os2")
                            for jj in range(2):
                                nc.scalar.copy(os2[:, jj, :], p3[jj])
                            for jj in range(2):
                                dc = dp + jj
                                nc.sync.dma_start(
                                    out=outT[dc * 128:(dc + 1) * 128,
                                             sbq * QTILE:(sbq + 1) * QTILE],
                                    in_=os2[:, jj, :])
                            continue
                        sb2 = ps_s.tile([128, 2, QTILE], F32, tag="s", name="sb2")
                        for jj in range(2):
                            dc = dp + jj
                            for j in range(HPC):
                                nc.tensor.matmul(sb2[:, jj, :],
                                                 wo_t[:, j, dc * 128:(dc + 1) * 128],
                                                 OT[:, j, sbq, :],
                                                 start=(j == 0), stop=(j == HPC - 1))
                        os2 = outs.tile([128, 2, QTILE], BF16, tag="os2")
                        nc.vector.tensor_copy(os2.rearrange("p a b -> p (a b)"),
                                              sb2.rearrange("p a b -> p (a b)"))
                        for jj in range(2):
                            dc = dp + jj
                            nc.sync.dma_start(
                                out=outT[dc * 128:(dc + 1) * 128,
                                         sbq * QTILE:(sbq + 1) * QTILE],
                                in_=os2[:, jj, :])

                def norm_chain(sbq, po, pr, tail=False):
                    # one chain per q-tile row covering both heads: DVE
                    # reciprocals -> K=1 matmul broadcast (f32r, into a
                    # rotating score-pool tile) -> fused po*rb -> OT multiply.
                    def _a():
                        rp = []
                        for h in range(HPC):
                            rpf = rbs.tile([1, QTILE], F32, tag=f"rpf{h}", name="rpf")
                            nc.vector.reciprocal_approx_fast(rpf, pr[h])
                            rph = rbs.tile([1, QTILE], F32R, tag=f"rp{h}", name="rp")
                            with nc.allow_low_precision(reason="fp32r bits are fp32"):
                                nc.vector.tensor_copy(rph, rpf)
                            rp.append(rph)

                        def _b():
                            rb_ps = ps_s.tile([128, 2, QTILE], F32, tag="s", name="rb_ps")
                            for h in range(HPC):
                                nc.tensor.matmul(rb_ps[:, h, :], onesr_t[:, :], rp[h][:, :],
                                                 start=True, stop=True)
                            rb = rbs.tile([128, 2, QTILE], F32, tag="rb", name="rb")
                            nc.vector.tensor_copy(rb.rearrange("p a b -> p (a b)"),
                                                  rb_ps.rearrange("p a b -> p (a b)"))

                            def _c():
                                for h in range(HPC):
                                    nc.vector.tensor_mul(OT[:, h, sbq, :], po[h], rb[:, h, :])
                                if tail:
                                    for w in range(4):
                                        sched(1 + w, lambda w=w: wo_part(sbq, w * 4, 4, tail=True))
                                else:
                                    for w in range(8):
                                        sched(1 + w, lambda w=w: wo_part(sbq, w * 2, 2))
                            sched(1, _c)
                        sched(1, _b)
                    sched(1, _a)

                bqt_order = [(b, qt) for qt in range(nqt) for b in range(B)]
                for bi, (b, qt) in enumerate(bqt_order):
                    is_last_row = bi == len(bqt_order) - 1
                    sbq = (b * S) // QTILE + qt
                    blist = blocks[qt]
                    nbl = len(blist)
                    po = [ps_o.tile([128, QTILE], F32, tag=f"po{h}", name=f"po{h}",
                                    bufs=1)
                          for h in range(HPC)]
                    pr = [ps_r.tile([1, QTILE], F32, tag=f"pr{h}", name=f"pr{h}",
                                    bufs=1)
                          for h in range(HPC)]
                    # pairs of k-blocks, the two heads interleaved: both
                    # heads' QK matmuls are emitted before either head's PV,
                    # so the PE streams head h1's QK while h0's exp is on
                    # ScalarE, and h0's PV lands right as its exp finishes.
                    for p0 in range(0, nbl, 2):
                        pair = blist[p0:p0 + 2]
                        pt_h = []
                        for h in range(HPC):
                            s_big = ps_s.tile([128, 2, QTILE], F32, tag="s", name="s_big")
                            for j, (kt, qoff, m) in enumerate(pair):
                                sbk = (b * S + kt * 128) // QTILE
                                ck = (kt * 128) % QTILE
                                nc.tensor.matmul(s_big[:, j, qoff:QTILE],
                                                 KT[:, h, sbk, ck:ck + 128],
                                                 QT[:, h, sbq, qoff:QTILE],
                                                 start=True, stop=True)
                            pt2 = pts.tile([128, 2, QTILE], BF16, tag="pt", name="pt2")
                            # one exp per block: PV(j0) only waits its own
                            # block's exp, halving the QK->PV chain latency
                            for j, (kt, qoff, m) in enumerate(pair):
                                nc.scalar.activation(
                                    pt2[:, j, qoff:QTILE], s_big[:, j, qoff:QTILE],
                                    mybir.ActivationFunctionType.Exp,
                                    scale=float(ISQ))
                            pt_h.append(pt2)
                        for h in range(HPC):
                            pt2 = pt_h[h]
                            for j, (kt, qoff, m) in enumerate(pair):
                                if m is not None:
                                    # on the (otherwise idle) GpSimd engine so
                                    # the mask never queues behind the os2
                                    # evacuation casts on DVE
                                    mid, mw = m
                                    nc.gpsimd.tensor_mul(
                                        pt2[:, j, qoff:qoff + mw],
                                        pt2[:, j, qoff:qoff + mw],
                                        masks_t[:, mid, 0:mw])
                            for j, (kt, qoff, m) in enumerate(pair):
                                i = p0 + j
                                gkt = (b * S) // 128 + kt
                                nc.tensor.matmul(po[h][:, qoff:QTILE],
                                                 VS[:, gkt, h * HD:(h + 1) * HD],
                                                 pt2[:, j, qoff:QTILE],
                                                 start=(i == 0), stop=(i == nbl - 1))
                                nc.tensor.matmul(pr[h][:, qoff:QTILE],
                                                 ones_t[:, :], pt2[:, j, qoff:QTILE],
                                                 start=(i == 0), stop=(i == nbl - 1))
                            gstep += 1
                            emit_due()
                    norm_chain(sbq, po, pr, tail=is_last_row)
                while todo:
                    gstep += 1
                    emit_due()
    nc.compile()
    return nc


# ---------------------------------------------------------------------------
# Host-side preparation
# ---------------------------------------------------------------------------

_CACHE = {}


def _classify_blocks(mask):
    """mask: additive [S, S] (q, k) -> (blocks, mask_tiles[128*nm, QTILE]).

    Per block, leading fully-masked q-columns are trimmed (qoff): the QK/PV/
    rowsum matmuls move only the live q-span.  The remaining masked span
    [qoff, qoff+mw) gets a multiplicative tile of width mw (for a causal mask
    every diagonal block reduces to ONE canonical [128,128] triangle)."""
    nqt, nkb = S // QTILE, S // KBLK
    mult = np.exp(np.minimum(mask, 0.0).astype(np.float64)).astype(np.float32)
    blocks = []
    tiles = []
    tile_index = {}
    for qt in range(nqt):
        row = []
        qs = slice(qt * QTILE, (qt + 1) * QTILE)
        for kt in range(nkb):
            ks = slice(kt * KBLK, (kt + 1) * KBLK)
            blk = mult[qs, ks].T  # [k, q]
            if not blk.any():
                continue
            if (blk == 1.0).all():
                row.append((kt, 0, None))
                continue
            col_any = blk.any(axis=0)
            col_ones = (blk == 1.0).all(axis=0)
            qoff = int(np.argmax(col_any))          # first non-dead column
            nz = np.nonzero(~col_ones)[0]
            mw = int(nz[-1]) + 1 - qoff             # masked span width
            assert mw >= 1
            sub = np.ascontiguousarray(blk[:, qoff:qoff + mw])
            key = sub.tobytes()
            if key not in tile_index:
                tile_index[key] = len(tiles)
                padded = np.ones((KBLK, QTILE), np.float32)
                padded[:, :mw] = sub
                tiles.append(padded)
            row.append((kt, qoff, (tile_index[key], mw)))
        assert row, "fully-masked q-tile row: softmax undefined in this kernel"
        blocks.append(row)
    if not tiles:
        tiles.append(np.zeros((KBLK, QTILE), np.float32))
    return blocks, np.concatenate(tiles, axis=0)


def _perm_even_odd():
    p = np.empty(HD, np.int64)
    p[:64] = np.arange(0, HD, 2)
    p[64:] = np.arange(1, HD, 2)
    return p


def _bf16(a):
    return np.ascontiguousarray(np.asarray(a, np.float32).astype(ml_dtypes.bfloat16))


def kernel(x, wq, wk, wv, wo, wq_A, wq_B, wk_A, wk_B, wv_A, wv_B,
           wo_A, wo_B, cos, sin, mask):
    x = np.asarray(x, np.float32)
    to64 = lambda a: np.asarray(a, np.float32).astype(np.float64)

    wq_f = (to64(wq) + SCALING * (to64(wq_A) @ to64(wq_B))).astype(np.float32)
    wk_f = (to64(wk) + SCALING * (to64(wk_A) @ to64(wk_B))).astype(np.float32)
    wv_f = (to64(wv) + SCALING * (to64(wv_A) @ to64(wv_B))).astype(np.float32)
    wo_f = (to64(wo) + SCALING * (to64(wo_A) @ to64(wo_B))).astype(np.float32)

    perm = _perm_even_odd()
    full_perm = np.concatenate([h * HD + perm for h in range(H)])
    wq_p = wq_f[:, full_perm]
    wk_p = wk_f[:, full_perm]

    cosT = np.ascontiguousarray(np.asarray(cos, np.float32).T)  # [64, S]
    sinT = np.ascontiguousarray(np.asarray(sin, np.float32).T)
    cos2 = np.concatenate([cosT, cosT], axis=0)  # [128, S]
    sin2 = np.concatenate([sinT, sinT], axis=0)

    m2d = np.asarray(mask, np.float32).reshape(S, S)
    blocks, mask_tiles = _classify_blocks(m2d)
    nm = mask_tiles.shape[0] // 128

    sig = (tuple(tuple(r) for r in blocks), nm)
    if sig not in _CACHE:
        _CACHE[sig] = build_kernel(blocks, nm, D // 128, SEQ // QTILE,
                                   S // QTILE, S // KBLK)
    nc = _CACHE[sig]

    xT = _bf16(x.reshape(SEQ, D).T)
    ones = np.ones((128, 1), ml_dtypes.bfloat16)
    onesrow = np.ones((1, 128), np.float32)

    in_maps = []
    for c in range(N_CORES):
        cols = slice(c * DCOL, (c + 1) * DCOL)
        in_maps.append(dict(
            xT=xT,
            wq=_bf16(wq_p[:, cols]),
            wk=_bf16(wk_p[:, cols]),
            wv=_bf16(wv_f[:, cols]),
            wo=_bf16(wo_f[cols, :]),
            cos2=cos2, sin2=sin2, masks=_bf16(mask_tiles),
            ones=ones, onesrow=onesrow,
        ))

    global _LAST
    res = run_bass_kernel_spmd(nc, in_maps, list(range(N_CORES)), trace=_TRACE)
    _LAST = res
    acc = np.zeros((D, SEQ), np.float32)
    for r in res.results:
        acc += r["outT"].astype(np.float32)
    return np.ascontiguousarray(acc.T).reshape(B, S, D)


_TRACE = False   # test harness can set kernel._TRACE = True to profile
_LAST = None     # last BassKernelResults (exec_time_ns when traced)
